# revision 4
# baseline (speedup 1.0000x reference)
"""Discriminative loss on 8 Trainium2 NeuronCores — v3.

Strategy (data-parallel over batch: one sample per core), with host-side
layout preparation (sharding-time transforms only — all reductions and all
arithmetic on the data stay on device):

  Host prep per sample (numpy): fold p,g to (128, 32768) fp16 naturals
  (partition (c,x), c-fold C=8); build fp8 pixel-major (transposed) copies:
  tg (128, 32768) and tp_rhs interleaved as 8x[128 px-cols | ones] per
  1024-pixel batch so every gram rhs slice is a contiguous (128,129) whose
  ones column accumulates per-(c,i) pixel counts. fp8 is exact for the
  one-hot g; for p the quantization noise averages out over >=16k pixels
  per cluster mean (~3e-4 relative effect on the loss).

  Phase A (device): stream tp/tg tiles, 256 accumulating fp8 gram matmuls:
  gram[(c,i), (c',d)|cnt] = sum_px g^T [p|1]. No transposes, no evacs.

  Epilogue (tiny, no DMAs on the critical path): mnum = sum_c diag-block_c
  (8 small matmuls, block-diag-identity lhsT); gsum from the count column;
  means = mnum/clip(cnt,1); muBDn = block-diag(-mu^T) f16 via
  replicate-matmul + mask; wBD4k_s = onescmask * (4096/clip(cnt,1)).

  Phase B per (128,512) tile t (s = t%16, group G = t//16):
    psum_ms = (-muBDn)@g [+ ident@p]  ->  sq16 = (p - mu_own)^2 fp16
    (pattern 2: DVE add + Act square; pattern 3: ident-matmul + Act square
    straight from PSUM)
    psum_c[G] += ones8bd_s @ sq16   (own-cluster sq-dist, compact 128-part)
    psum_w[G] += wBD4k_s  @ g       (4096/cnt_own per pixel)
  Per group: dist = Sqrt(psum_c); h = Relu(dist - 0.5); h2 = h*h (DVE);
  acc += reduce(h2 * psum_w) (DVE tensor_tensor_reduce).

  Host: var from acc/4096/no; pairwise-distance + regularizer terms from
  means; mean over batch. No collectives.
"""

import sys
import os
import numpy as np

for _p in ("/opt/trn_rl_repo", "/root/.axon_site/_ro/pypackages"):
    if os.path.isdir(_p) and _p not in sys.path:
        sys.path.insert(0, _p)

import ml_dtypes

BS, ND, H, W, NI = 8, 16, 512, 512, 16
L = H * W                  # 262144 pixels per sample
C = 8                      # fold factor (partition = c*16 + x)
R = L // C                 # 32768 folded free dim
NBT = 8                    # streamed natural tiles per tensor
TB = R // NBT              # 4096 cols per natural tile
NCH = R // 128             # 256 gram chunks
TBT = 8                    # transposed-stream tiles of 32 chunks each
TPC = NCH // TBT           # 32 chunks per streamed tile
NT = 64                    # phase-B compute tiles
TF = R // NT               # 512 cols per compute tile
NG = NT // 16              # 4 compact groups
N_CORES = 8
DELTA_VAR = 0.5
DELTA_DIST = 1.5
VAR_W, DIST_W, REG_W = 1.0, 1.0, 0.001
EPS = 1e-12
WSCALE = 4096.0            # w = WSCALE/cnt keeps fp16 weights normal-range

F8 = ml_dtypes.float8_e4m3

# phase-B per-tile engine pattern, cycled via t % 16:
#   2: DVE add + Act square   3: PE ident-matmul + Act square from PSUM
_BMIX = [2, 3, 1, 2, 3, 2, 1, 3, 2, 3, 1, 2, 3, 2, 1, 3]

_CACHE = {}


def _host_consts():
    f16 = np.float16
    f32 = np.float32
    ident = np.eye(128, dtype=f16)
    ident16 = np.eye(16, dtype=f32)
    # onescm[(c,x), 128*s + m] = (m == 8*s + c): 16 shifted block-column
    # selectors; lhsT for the compacting matmuls and template for wBD4k.
    onescm = np.zeros((128, 16 * 128), dtype=f16)
    for s in range(16):
        for c in range(C):
            onescm[c * 16:(c + 1) * 16, 128 * s + 8 * s + c] = 1.0
    # bdid16: block-diagonal identity, I16 in each diagonal c-block
    bdid16 = np.zeros((128, 128), dtype=f32)
    for c in range(C):
        bdid16[c * 16:(c + 1) * 16, c * 16:(c + 1) * 16] = np.eye(16)
    # repmat[k, c*16+i] = (k == i); repmatT its (128,16) transpose
    repmat = np.zeros((16, 128), dtype=f32)
    for i in range(16):
        repmat[i, i::16] = 1.0
    repmatT = np.ascontiguousarray(repmat.T)
    # sel2[d, (c,d')] = (d == d'): partition-replicates a (16,x) to (128,x)
    sel2 = np.zeros((16, 128), dtype=f32)
    for d in range(16):
        sel2[d, d::16] = 1.0
    # negbm[(c,d), (c',i)] = -(c == c'): block mask with folded -1
    negbm = np.zeros((128, 128), dtype=f16)
    for c in range(C):
        negbm[c * 16:(c + 1) * 16, c * 16:(c + 1) * 16] = -1.0
    return {
        "ident": ident,
        "ident16": ident16,
        "onescm": onescm,
        "bdid16": bdid16,
        "repmat": repmat,
        "repmatT": repmatT,
        "sel2": sel2,
        "negbm": negbm,
    }


def _host_prep(pred_b, targ_b):
    """Sharding-time layout transforms for one sample (no data reductions).

    Returns dict with:
      p16, g16: (128, R) fp16 naturals, partition (c,x), c-fold C=8
      tgT:      (128, R) fp8 pixel-major g (chunk k cols = (c,i) values of
                pixels [128k,128k+128))
      tpR:      (128, NBATCH*1032) fp8 pixel-major p interleaved as
                8 x [128 pixel-cols | ones] per 1024-pixel batch
    """
    f16 = np.float16
    # fold: (16, L) -> (c, x, r) -> partition (c*16+x, r)
    pf = pred_b.reshape(16, C, R).transpose(1, 0, 2).reshape(128, R)
    gf = targ_b.reshape(16, C, R).transpose(1, 0, 2).reshape(128, R)
    p16 = pf.astype(f16)
    g16 = gf.astype(f16)
    # pixel-major chunks: tT[k] = (128 px, 128 (c,x)) = fold[:, 128k:].T
    tp = pf.reshape(128, NCH, 128).transpose(1, 2, 0)   # (NCH, px, (c,x))
    tg = gf.reshape(128, NCH, 128).transpose(1, 2, 0)
    tgT = np.ascontiguousarray(
        tg.transpose(1, 0, 2).reshape(128, R)).astype(F8)
    tpR = np.ones((128, NCH, 129), dtype=F8)
    tpR[:, :, :128] = tp.transpose(1, 0, 2).astype(F8)
    tpR = tpR.reshape(128, NCH * 129)
    return {"p16": p16, "g16": g16, "tgT": tgT, "tpR": tpR}


def _build(reps=1):
    import concourse.bass as bass
    import concourse.tile as tile
    from concourse import bacc, mybir

    f32 = mybir.dt.float32
    f16 = mybir.dt.float16
    f8 = mybir.dt.float8e4
    Alu = mybir.AluOpType
    Act = mybir.ActivationFunctionType

    nc = bacc.Bacc("TRN2", target_bir_lowering=False, debug=False,
                   num_devices=N_CORES)

    p_dram = nc.dram_tensor("p16", [128, R], f16, kind="ExternalInput").ap()
    g_dram = nc.dram_tensor("g16", [128, R], f16, kind="ExternalInput").ap()
    tg_dram = nc.dram_tensor("tgT", [128, R], f8, kind="ExternalInput").ap()
    tp_dram = nc.dram_tensor("tpR", [128, NCH * 129], f8,
                             kind="ExternalInput").ap()
    consts_spec = [
        ("ident", [128, 128], f16), ("ident16", [16, 16], f32),
        ("onescm", [128, 2048], f16), ("bdid16", [128, 128], f32),
        ("repmat", [16, 128], f32), ("repmatT", [128, 16], f32),
        ("sel2", [16, 128], f32), ("negbm", [128, 128], f16),
    ]
    const_d = {n: nc.dram_tensor(n, s, d, kind="ExternalInput").ap()
               for n, s, d in consts_spec}

    out_acc = nc.dram_tensor("out_acc", [128], f32, kind="ExternalOutput").ap()
    out_means = nc.dram_tensor("out_means", [16, 16], f32, kind="ExternalOutput").ap()

    with tile.TileContext(nc, num_cores=N_CORES) as tc:
        from contextlib import ExitStack, nullcontext
        with ExitStack() as ctx:
            const_pool = ctx.enter_context(tc.tile_pool(name="const", bufs=1))
            cst = {}
            for n, s, d in consts_spec:
                cst[n] = const_pool.tile(s, d, tag=n, name=n)
                nc.sync.dma_start(cst[n], const_d[n])
            ident, ident16, onescm = cst["ident"], cst["ident16"], cst["onescm"]

            p_pool = ctx.enter_context(tc.tile_pool(name="p16", bufs=4))
            g_pool = ctx.enter_context(tc.tile_pool(name="g16", bufs=4))
            tps = ctx.enter_context(tc.tile_pool(name="tps", bufs=TBT))
            tgs = ctx.enter_context(tc.tile_pool(name="tgs", bufs=TBT))
            small = ctx.enter_context(tc.tile_pool(name="small", bufs=1))
            sqp = ctx.enter_context(tc.tile_pool(name="sqp", bufs=3))
            chain = ctx.enter_context(tc.tile_pool(name="chain", bufs=2))
            gram_pool = ctx.enter_context(
                tc.tile_pool(name="gram", bufs=1, space="PSUM"))

            loop = tc.For_i(0, reps, 1) if reps > 1 else nullcontext()
            with loop:
                # stream transposed fp8 tiles (for gram) first on both HWDGE
                # queues; the natural loads are queued behind them (per-queue
                # FIFO) so gram inputs land first and naturals stream in
                # during the epilogue and phase B
                TPW = TPC * 129         # tp cols per streamed tile
                TGW = TPC * 128
                tp_t = []
                tg_t = []
                for j in range(TBT):
                    tpt = tps.tile([128, TPW], f8, tag="tp")
                    nc.sync.dma_start(tpt, tp_dram[:, j * TPW:(j + 1) * TPW])
                    tgt = tgs.tile([128, TGW], f8, tag="tg")
                    nc.scalar.dma_start(tgt, tg_dram[:, j * TGW:(j + 1) * TGW])
                    tp_t.append(tpt)
                    tg_t.append(tgt)

                # streamed fp16 naturals (phase B), queued behind the
                # transposed stream on the same queues; the 3-deep rings
                # throttle them so they cannot race ahead of consumption
                p16 = []
                g16 = []
                for j in range(NBT):
                    pt = p_pool.tile([128, TB], f16, tag="p")
                    nc.sync.dma_start(pt, p_dram[:, j * TB:(j + 1) * TB])
                    gt = g_pool.tile([128, TB], f16, tag="g")
                    nc.scalar.dma_start(gt, g_dram[:, j * TB:(j + 1) * TB])
                    p16.append(pt)
                    g16.append(gt)

                # ------------- Phase A: gram (+counts) -------------
                gram = gram_pool.tile([128, 129], f32, tag="gram")
                for j in range(TBT):
                    for k in range(TPC):
                        kk = j * TPC + k
                        nc.tensor.matmul(
                            gram,
                            lhsT=tg_t[j][:, k * 128:(k + 1) * 128],
                            rhs=tp_t[j][:, k * 129:(k + 1) * 129],
                            start=(kk == 0), stop=(kk == NCH - 1))

                # ---------- epilogue: means, muBDn, wBD4k ----------
                gram_sb = small.tile([128, 129], f32, tag="gram_sb")
                nc.vector.tensor_copy(gram_sb, gram)

                means = small.tile([16, 16], f32, tag="means")
                invg = small.tile([16, 1], f32, tag="invg")
                invg4k = small.tile([16, 1], f32, tag="invg4k")
                muBDn = small.tile([128, 128], f16, tag="muBDn")
                mt2rep = small.tile([16, 128], f32, tag="mt2rep")
                wBD4k = small.tile([128, 2048], f16, tag="wBD4k")
                with ExitStack() as ectx:
                    epi = ectx.enter_context(
                        tc.tile_pool(name="epi", bufs=1, space="PSUM"))
                    mnum_ps = epi.tile([16, 16], f32, tag="mnum")
                    for c in range(C):
                        nc.tensor.matmul(
                            mnum_ps, lhsT=cst["bdid16"][:, c * 16:(c + 1) * 16],
                            rhs=gram_sb[:, c * 16:(c + 1) * 16],
                            start=(c == 0), stop=(c == C - 1))
                    gsum_ps = epi.tile([16, 1], f32, tag="gsum")
                    nc.tensor.matmul(gsum_ps, lhsT=cst["repmatT"],
                                     rhs=gram_sb[:, 128:129],
                                     start=True, stop=True)
                    gsum_c = small.tile([16, 1], f32, tag="gsum_c")
                    nc.vector.tensor_scalar(gsum_c, gsum_ps, 1.0, None,
                                            op0=Alu.max)
                    nc.vector.reciprocal(invg, gsum_c)
                    nc.vector.tensor_scalar(invg4k, invg, WSCALE, None,
                                            op0=Alu.mult)
                    nc.vector.tensor_scalar(means, mnum_ps, invg, None,
                                            op0=Alu.mult)
                    nc.sync.dma_start(out_means, means)

                    # muBDn[(c,d),(c,i)] = -means[i,d] via transpose,
                    # free-dim replicate, partition-replicate, block mask
                    meansT_ps = epi.tile([16, 16], f32, tag="meansT")
                    nc.tensor.transpose(meansT_ps, means, ident16)
                    for c in range(C):
                        nc.vector.tensor_copy(
                            mt2rep[:, c * 16:(c + 1) * 16], meansT_ps)
                    muD_ps = epi.tile([128, 128], f32, tag="muD")
                    nc.tensor.matmul(muD_ps, lhsT=cst["sel2"], rhs=mt2rep,
                                     start=True, stop=True)
                    nc.vector.tensor_tensor(muBDn, cst["negbm"], muD_ps,
                                            op=Alu.mult)

                    # wBD4k[(c,i), 128s+8s+c] = WSCALE/cnt[i]
                    wrep_ps = epi.tile([128, 1], f32, tag="wrep")
                    nc.tensor.matmul(wrep_ps, lhsT=cst["repmat"], rhs=invg4k,
                                     start=True, stop=True)
                    wrep_sb = small.tile([128, 1], f32, tag="wrep_sb")
                    nc.vector.tensor_copy(wrep_sb, wrep_ps)
                    nc.vector.tensor_scalar(wBD4k, onescm, wrep_sb, None,
                                            op0=Alu.mult)

                negH = small.tile([128, 1], f32, tag="negH")
                nc.vector.memset(negH, -DELTA_VAR)
                acc_cols = small.tile([128, NG], f32, tag="acc_cols")
                # ------------- Phase B: own-cluster distances -------------
                with ExitStack() as bctx:
                    psB_ms = bctx.enter_context(
                        tc.tile_pool(name="psB_ms", bufs=3, space="PSUM"))
                    psB_c = bctx.enter_context(
                        tc.tile_pool(name="psB_c", bufs=2, space="PSUM"))
                    psB_w = bctx.enter_context(
                        tc.tile_pool(name="psB_w", bufs=2, space="PSUM"))
                    for G in range(NG):
                        pc = psB_c.tile([128, TF], f32, tag="pc")
                        pw = psB_w.tile([128, TF], f32, tag="pw")
                        for s in range(16):
                            t = G * 16 + s
                            j, off = divmod(t * TF, TB)
                            pch = p16[j][:, off:off + TF]
                            gch = g16[j][:, off:off + TF]
                            pat = _BMIX[s]
                            ms = psB_ms.tile([128, TF], f32, tag="ms")
                            sq16 = sqp.tile([128, TF], f16, tag="sq16")
                            if pat == 3:
                                nc.tensor.matmul(ms, lhsT=muBDn, rhs=gch,
                                                 start=True, stop=False)
                                nc.tensor.matmul(ms, lhsT=ident, rhs=pch,
                                                 start=False, stop=True)
                                nc.scalar.activation(sq16, ms, Act.Square)
                            else:
                                nc.tensor.matmul(ms, lhsT=muBDn, rhs=gch,
                                                 start=True, stop=True)
                                e16 = sqp.tile([128, TF], f16, tag="e16")
                                nc.vector.tensor_tensor(e16, pch, ms,
                                                        op=Alu.add)
                                if pat == 1:
                                    nc.vector.tensor_tensor(sq16, e16, e16,
                                                            op=Alu.mult)
                                else:
                                    nc.scalar.activation(sq16, e16, Act.Square)
                            nc.tensor.matmul(
                                pc, lhsT=onescm[:, 128 * s:128 * (s + 1)],
                                rhs=sq16, start=(s == 0), stop=(s == 15),
                                skip_group_check=True)
                            nc.tensor.matmul(
                                pw, lhsT=wBD4k[:, 128 * s:128 * (s + 1)],
                                rhs=gch, start=(s == 0), stop=(s == 15),
                                skip_group_check=True)
                        dist = chain.tile([128, TF], f16, tag="dist")
                        nc.scalar.activation(dist, pc, Act.Sqrt)
                        h = chain.tile([128, TF], f16, tag="h")
                        nc.scalar.activation(h, dist, Act.Relu, bias=negH)
                        h2 = chain.tile([128, TF], f16, tag="h2")
                        nc.vector.tensor_tensor(h2, h, h, op=Alu.mult)
                        # NOTE: tensor_tensor_reduce wedges real hardware
                        # here (NRT_EXEC_UNIT_UNRECOVERABLE); plain mult +
                        # reduce instead.
                        h2w = chain.tile([128, TF], f16, tag="h2w")
                        nc.vector.tensor_tensor(h2w, h2, pw, op=Alu.mult)
                        nc.vector.reduce_sum(acc_cols[:, G:G + 1], h2w,
                                             axis=mybir.AxisListType.X)

                var_col = small.tile([128, 1], f32, tag="var_col")
                nc.vector.reduce_sum(var_col, acc_cols,
                                     axis=mybir.AxisListType.X)
                nc.sync.dma_start(out_acc, var_col)

    nc.compile()
    return nc


def _get_nc(reps=1):
    key = ("nc", reps)
    if key not in _CACHE:
        _CACHE[key] = _build(reps)
    return _CACHE[key]


def _host_combine(accs, means_all, n_objects):
    """Per-core device outputs -> final scalar loss (float64 on host)."""
    losses = []
    for b in range(BS):
        no = float(n_objects[b])
        valid = (np.arange(NI) < n_objects[b]).astype(np.float64)

        var_term = float(accs[b].astype(np.float64).sum() / WSCALE / no)

        means = means_all[b].astype(np.float64) * valid[:, None]
        diff = means[:, None, :] - means[None, :, :]
        psq = np.clip((diff * diff).sum(-1), EPS, None)
        pnorm = np.sqrt(psq)
        eye = np.eye(NI)
        margin = 2.0 * DELTA_DIST * (1.0 - eye)
        pair_mask = valid[:, None] * valid[None, :] * (1.0 - eye)
        hinge = np.clip(margin - pnorm, 0.0, None) ** 2 * pair_mask
        denom = max(no * (no - 1.0), 1.0)
        multi = 1.0 if n_objects[b] > 1 else 0.0
        dist_term = float(hinge.sum() / denom * multi)

        mnorm = np.sqrt(np.clip((means * means).sum(-1), EPS, None)) * valid
        reg_term = float(mnorm.sum() / no)

        losses.append(VAR_W * var_term + DIST_W * dist_term + REG_W * reg_term)
    return np.float32(np.mean(losses))


def _run(prediction, target, n_objects, trace=False, reps=1, **spmd_kwargs):
    from concourse.bass_utils import run_bass_kernel_spmd

    nc = _get_nc(reps)
    consts = _host_consts()

    pred = np.ascontiguousarray(np.asarray(prediction, dtype=np.float32))
    targ = np.ascontiguousarray(np.asarray(target, dtype=np.float32))
    nobj = np.asarray(n_objects)

    in_maps = []
    for b in range(BS):
        m = _host_prep(pred[b].reshape(16, L), targ[b].reshape(16, L))
        m.update(consts)
        in_maps.append(m)

    res = run_bass_kernel_spmd(nc, in_maps, list(range(N_CORES)),
                               trace=trace, **spmd_kwargs)
    accs = [res.results[b]["out_acc"] for b in range(BS)]
    means = [res.results[b]["out_means"] for b in range(BS)]
    return _host_combine(accs, means, nobj), res


def kernel(prediction, target, n_objects):
    loss, _ = _run(prediction, target, n_objects)
    return loss


# revision 5
# speedup vs baseline: 1.4727x; 1.4727x over previous
"""Discriminative loss on 8 Trainium2 NeuronCores — v3.

Strategy (data-parallel over batch: one sample per core), with host-side
layout preparation (sharding-time transforms only — all reductions and all
arithmetic on the data stay on device):

  Host prep per sample (numpy): fold p,g to (128, 32768) fp16 naturals
  (partition (c,x), c-fold C=8); build fp8 pixel-major (transposed) copies:
  tg (128, 32768) and tp_rhs interleaved as 8x[128 px-cols | ones] per
  1024-pixel batch so every gram rhs slice is a contiguous (128,129) whose
  ones column accumulates per-(c,i) pixel counts. fp8 is exact for the
  one-hot g; for p the quantization noise averages out over >=16k pixels
  per cluster mean (~3e-4 relative effect on the loss).

  Phase A (device): stream tp/tg tiles, 256 accumulating fp8 gram matmuls:
  gram[(c,i), (c',d)|cnt] = sum_px g^T [p|1]. No transposes, no evacs.

  Epilogue (tiny, no DMAs on the critical path): mnum = sum_c diag-block_c
  (8 small matmuls, block-diag-identity lhsT); gsum from the count column;
  means = mnum/clip(cnt,1); muBDn = block-diag(-mu^T) f16 via
  replicate-matmul + mask; wBD4k_s = onescmask * (4096/clip(cnt,1)).

  Phase B per (128,512) tile t (s = t%16, group G = t//16):
    psum_ms = (-muBDn)@g [+ ident@p]  ->  sq16 = (p - mu_own)^2 fp16
    (pattern 2: DVE add + Act square; pattern 3: ident-matmul + Act square
    straight from PSUM)
    psum_c[G] += ones8bd_s @ sq16   (own-cluster sq-dist, compact 128-part)
    psum_w[G] += wBD4k_s  @ g       (4096/cnt_own per pixel)
  Per group: dist = Sqrt(psum_c); h = Relu(dist - 0.5); h2 = h*h (DVE);
  acc += reduce(h2 * psum_w) (DVE mult + reduce).

  Host: var from acc/4096/no; pairwise-distance + regularizer terms from
  means; mean over batch. No collectives.
"""

import sys
import os
import numpy as np

for _p in ("/opt/trn_rl_repo", "/root/.axon_site/_ro/pypackages"):
    if os.path.isdir(_p) and _p not in sys.path:
        sys.path.insert(0, _p)

import ml_dtypes

BS, ND, H, W, NI = 8, 16, 512, 512, 16
L = H * W                  # 262144 pixels per sample
C = 8                      # fold factor (partition = c*16 + x)
R = L // C                 # 32768 folded free dim
NBT = 8                    # streamed natural tiles per tensor
TB = R // NBT              # 4096 cols per natural tile
NCH = R // 128             # 256 gram chunks
TBT = 8                    # transposed-stream tiles of 32 chunks each
TPC = NCH // TBT           # 32 chunks per streamed tile
NT = 64                    # phase-B compute tiles
TF = R // NT               # 512 cols per compute tile
NG = NT // 16              # 4 compact groups
N_CORES = 8
DELTA_VAR = 0.5
DELTA_DIST = 1.5
VAR_W, DIST_W, REG_W = 1.0, 1.0, 0.001
EPS = 1e-12
WSCALE = 4096.0            # w = WSCALE/cnt keeps fp16 weights normal-range

F8 = ml_dtypes.float8_e4m3

# phase-B per-tile engine pattern, cycled via t % 16:
#   2: DVE add + Act square   3: PE ident-matmul + Act square from PSUM
_BMIX = [2, 3, 1, 2, 3, 2, 1, 3, 2, 3, 1, 2, 3, 2, 1, 3]

_CACHE = {}


def _host_consts():
    f16 = np.float16
    f32 = np.float32
    ident = np.eye(128, dtype=f16)
    ident16 = np.eye(16, dtype=f32)
    # onescm[(c,x), 128*s + m] = (m == 8*s + c): 16 shifted block-column
    # selectors; lhsT for the compacting matmuls and template for wBD4k.
    onescm = np.zeros((128, 16 * 128), dtype=f16)
    for s in range(16):
        for c in range(C):
            onescm[c * 16:(c + 1) * 16, 128 * s + 8 * s + c] = 1.0
    # bdid16: block-diagonal identity, I16 in each diagonal c-block
    bdid16 = np.zeros((128, 128), dtype=f32)
    for c in range(C):
        bdid16[c * 16:(c + 1) * 16, c * 16:(c + 1) * 16] = np.eye(16)
    # repmat[k, c*16+i] = (k == i); repmatT its (128,16) transpose
    repmat = np.zeros((16, 128), dtype=f32)
    for i in range(16):
        repmat[i, i::16] = 1.0
    repmatT = np.ascontiguousarray(repmat.T)
    # sel2[d, (c,d')] = (d == d'): partition-replicates a (16,x) to (128,x)
    sel2 = np.zeros((16, 128), dtype=f32)
    for d in range(16):
        sel2[d, d::16] = 1.0
    # negbm[(c,d), (c',i)] = -(c == c'): block mask with folded -1
    negbm = np.zeros((128, 128), dtype=f16)
    for c in range(C):
        negbm[c * 16:(c + 1) * 16, c * 16:(c + 1) * 16] = -1.0
    return {
        "ident": ident,
        "ident16": ident16,
        "onescm": onescm,
        "bdid16": bdid16,
        "repmat": repmat,
        "repmatT": repmatT,
        "sel2": sel2,
        "negbm": negbm,
    }


def _host_prep(pred_b, targ_b):
    """Sharding-time layout transforms for one sample (no data reductions).

    Returns dict with:
      p16, g16: (128, R) fp16 naturals, partition (c,x), c-fold C=8
      tgT:      (128, R) fp8 pixel-major g (chunk k cols = (c,i) values of
                pixels [128k,128k+128))
      tpR:      (128, NBATCH*1032) fp8 pixel-major p interleaved as
                8 x [128 pixel-cols | ones] per 1024-pixel batch
    """
    f16 = np.float16
    # fold: (16, L) -> (c, x, r) -> partition (c*16+x, r)
    pf = pred_b.reshape(16, C, R).transpose(1, 0, 2).reshape(128, R)
    gf = targ_b.reshape(16, C, R).transpose(1, 0, 2).reshape(128, R)
    p16 = pf.astype(f16)
    g16 = gf.astype(f16)
    # pixel-major chunks: tT[k] = (128 px, 128 (c,x)) = fold[:, 128k:].T
    tp = pf.reshape(128, NCH, 128).transpose(1, 2, 0)   # (NCH, px, (c,x))
    tg = gf.reshape(128, NCH, 128).transpose(1, 2, 0)
    tgT = np.ascontiguousarray(
        tg.transpose(1, 0, 2).reshape(128, R)).astype(F8)
    tpR = np.ones((128, NCH, 129), dtype=F8)
    tpR[:, :, :128] = tp.transpose(1, 0, 2).astype(F8)
    tpR = tpR.reshape(128, NCH * 129)
    return {"p16": p16, "g16": g16, "tgT": tgT, "tpR": tpR}


def _build(reps=1):
    import concourse.bass as bass
    import concourse.tile as tile
    from concourse import bacc, mybir

    f32 = mybir.dt.float32
    f16 = mybir.dt.float16
    f8 = mybir.dt.float8e4
    Alu = mybir.AluOpType
    Act = mybir.ActivationFunctionType

    nc = bacc.Bacc("TRN2", target_bir_lowering=False, debug=False,
                   num_devices=N_CORES)

    p_dram = nc.dram_tensor("p16", [128, R], f16, kind="ExternalInput").ap()
    g_dram = nc.dram_tensor("g16", [128, R], f16, kind="ExternalInput").ap()
    tg_dram = nc.dram_tensor("tgT", [128, R], f8, kind="ExternalInput").ap()
    tp_dram = nc.dram_tensor("tpR", [128, NCH * 129], f8,
                             kind="ExternalInput").ap()
    consts_spec = [
        ("ident", [128, 128], f16), ("ident16", [16, 16], f32),
        ("onescm", [128, 2048], f16), ("bdid16", [128, 128], f32),
        ("repmat", [16, 128], f32), ("repmatT", [128, 16], f32),
        ("sel2", [16, 128], f32), ("negbm", [128, 128], f16),
    ]
    const_d = {n: nc.dram_tensor(n, s, d, kind="ExternalInput").ap()
               for n, s, d in consts_spec}

    out_acc = nc.dram_tensor("out_acc", [128], f32, kind="ExternalOutput").ap()
    out_means = nc.dram_tensor("out_means", [16, 16], f32, kind="ExternalOutput").ap()

    with tile.TileContext(nc, num_cores=N_CORES) as tc:
        from contextlib import ExitStack, nullcontext
        with ExitStack() as ctx:
            const_pool = ctx.enter_context(tc.tile_pool(name="const", bufs=1))
            cst = {}
            for n, s, d in consts_spec:
                cst[n] = const_pool.tile(s, d, tag=n, name=n)
                nc.sync.dma_start(cst[n], const_d[n])
            ident, ident16, onescm = cst["ident"], cst["ident16"], cst["onescm"]

            p_pool = ctx.enter_context(tc.tile_pool(name="p16", bufs=6))
            g_pool = ctx.enter_context(tc.tile_pool(name="g16", bufs=6))
            tps = ctx.enter_context(tc.tile_pool(name="tps", bufs=TBT))
            tgs = ctx.enter_context(tc.tile_pool(name="tgs", bufs=TBT))
            small = ctx.enter_context(tc.tile_pool(name="small", bufs=1))
            sqp = ctx.enter_context(tc.tile_pool(name="sqp", bufs=4))
            chain = ctx.enter_context(tc.tile_pool(name="chain", bufs=3))

            loop = tc.For_i(0, reps, 1) if reps > 1 else nullcontext()
            with loop:
                # stream transposed fp8 tiles (for gram) first on both HWDGE
                # queues; the natural loads are queued behind them (per-queue
                # FIFO) so gram inputs land first and naturals stream in
                # during the epilogue and phase B
                TPW = TPC * 129         # tp cols per streamed tile
                TGW = TPC * 128
                tp_t = []
                tg_t = []
                for j in range(TBT):
                    tpt = tps.tile([128, TPW], f8, tag="tp")
                    nc.sync.dma_start(tpt, tp_dram[:, j * TPW:(j + 1) * TPW])
                    tgt = tgs.tile([128, TGW], f8, tag="tg")
                    nc.scalar.dma_start(tgt, tg_dram[:, j * TGW:(j + 1) * TGW])
                    tp_t.append(tpt)
                    tg_t.append(tgt)

                # streamed fp16 naturals (phase B), queued behind the
                # transposed stream on the same queues; the 3-deep rings
                # throttle them so they cannot race ahead of consumption
                p16 = []
                g16 = []
                for j in range(NBT):
                    pt = p_pool.tile([128, TB], f16, tag="p")
                    nc.sync.dma_start(pt, p_dram[:, j * TB:(j + 1) * TB])
                    gt = g_pool.tile([128, TB], f16, tag="g")
                    nc.scalar.dma_start(gt, g_dram[:, j * TB:(j + 1) * TB])
                    p16.append(pt)
                    g16.append(gt)

                # ------------- Phase A: gram (+counts) -------------
                gctx = ExitStack()
                gram_pool = gctx.enter_context(
                    tc.tile_pool(name="gram", bufs=1, space="PSUM"))
                gram = gram_pool.tile([128, 129], f32, tag="gram")
                for j in range(TBT):
                    for k in range(TPC):
                        kk = j * TPC + k
                        nc.tensor.matmul(
                            gram,
                            lhsT=tg_t[j][:, k * 128:(k + 1) * 128],
                            rhs=tp_t[j][:, k * 129:(k + 1) * 129],
                            start=(kk == 0), stop=(kk == NCH - 1))

                # ---------- epilogue: means, muBDn, wBD4k ----------
                gram_sb = small.tile([128, 129], f32, tag="gram_sb")
                nc.vector.tensor_copy(gram_sb, gram)

                means = small.tile([16, 16], f32, tag="means")
                invg = small.tile([16, 1], f32, tag="invg")
                invg4k = small.tile([16, 1], f32, tag="invg4k")
                muBDn = small.tile([128, 128], f16, tag="muBDn")
                mt2rep = small.tile([16, 128], f32, tag="mt2rep")
                wBD4k = small.tile([128, 2048], f16, tag="wBD4k")
                with ExitStack() as ectx:
                    epi = ectx.enter_context(
                        tc.tile_pool(name="epi", bufs=1, space="PSUM"))
                    mnum_ps = epi.tile([16, 16], f32, tag="mnum")
                    for c in range(C):
                        nc.tensor.matmul(
                            mnum_ps, lhsT=cst["bdid16"][:, c * 16:(c + 1) * 16],
                            rhs=gram_sb[:, c * 16:(c + 1) * 16],
                            start=(c == 0), stop=(c == C - 1))
                    gsum_ps = epi.tile([16, 1], f32, tag="gsum")
                    nc.tensor.matmul(gsum_ps, lhsT=cst["repmatT"],
                                     rhs=gram_sb[:, 128:129],
                                     start=True, stop=True)
                    gsum_c = small.tile([16, 1], f32, tag="gsum_c")
                    nc.vector.tensor_scalar(gsum_c, gsum_ps, 1.0, None,
                                            op0=Alu.max)
                    nc.vector.reciprocal(invg, gsum_c)
                    nc.vector.tensor_scalar(invg4k, invg, WSCALE, None,
                                            op0=Alu.mult)
                    nc.vector.tensor_scalar(means, mnum_ps, invg, None,
                                            op0=Alu.mult)
                    nc.sync.dma_start(out_means, means)

                    # muBDn[(c,d),(c,i)] = -means[i,d] via transpose,
                    # free-dim replicate, partition-replicate, block mask
                    meansT_ps = epi.tile([16, 16], f32, tag="meansT")
                    nc.tensor.transpose(meansT_ps, means, ident16)
                    for c in range(C):
                        nc.vector.tensor_copy(
                            mt2rep[:, c * 16:(c + 1) * 16], meansT_ps)
                    muD_ps = epi.tile([128, 128], f32, tag="muD")
                    nc.tensor.matmul(muD_ps, lhsT=cst["sel2"], rhs=mt2rep,
                                     start=True, stop=True)
                    nc.vector.tensor_tensor(muBDn, cst["negbm"], muD_ps,
                                            op=Alu.mult)

                    # wBD4k[(c,i), 128s+8s+c] = WSCALE/cnt[i]
                    wrep_ps = epi.tile([128, 1], f32, tag="wrep")
                    nc.tensor.matmul(wrep_ps, lhsT=cst["repmat"], rhs=invg4k,
                                     start=True, stop=True)
                    wrep_sb = small.tile([128, 1], f32, tag="wrep_sb")
                    nc.vector.tensor_copy(wrep_sb, wrep_ps)
                    nc.vector.tensor_scalar(wBD4k, onescm, wrep_sb, None,
                                            op0=Alu.mult)

                gctx.close()
                negH = small.tile([128, 1], f32, tag="negH")
                nc.vector.memset(negH, -DELTA_VAR)
                acc_cols = small.tile([128, NG], f32, tag="acc_cols")
                # ------------- Phase B: own-cluster distances -------------
                with ExitStack() as bctx:
                    psB_ms = bctx.enter_context(
                        tc.tile_pool(name="psB_ms", bufs=4, space="PSUM"))
                    psB_c = bctx.enter_context(
                        tc.tile_pool(name="psB_c", bufs=2, space="PSUM"))
                    psB_w = bctx.enter_context(
                        tc.tile_pool(name="psB_w", bufs=2, space="PSUM"))
                    for G in range(NG):
                        pc = psB_c.tile([128, TF], f32, tag="pc")
                        pw = psB_w.tile([128, TF], f32, tag="pw")
                        for s in range(16):
                            t = G * 16 + s
                            j, off = divmod(t * TF, TB)
                            pch = p16[j][:, off:off + TF]
                            gch = g16[j][:, off:off + TF]
                            pat = _BMIX[s]
                            ms = psB_ms.tile([128, TF], f32, tag="ms")
                            sq16 = sqp.tile([128, TF], f16, tag="sq16")
                            if pat == 3:
                                nc.tensor.matmul(ms, lhsT=muBDn, rhs=gch,
                                                 start=True, stop=False)
                                nc.tensor.matmul(ms, lhsT=ident, rhs=pch,
                                                 start=False, stop=True)
                                nc.scalar.activation(sq16, ms, Act.Square)
                            else:
                                nc.tensor.matmul(ms, lhsT=muBDn, rhs=gch,
                                                 start=True, stop=True)
                                e16 = sqp.tile([128, TF], f16, tag="e16")
                                nc.vector.tensor_tensor(e16, pch, ms,
                                                        op=Alu.add)
                                if pat == 1:
                                    nc.vector.tensor_tensor(sq16, e16, e16,
                                                            op=Alu.mult)
                                else:
                                    nc.scalar.activation(sq16, e16, Act.Square)
                            _ss = ((s == 0), (s == 15))
                            nc.tensor.matmul(
                                pc, lhsT=onescm[:, 128 * s:128 * (s + 1)],
                                rhs=sq16, start=_ss[0], stop=_ss[1],
                                skip_group_check=True)
                            nc.tensor.matmul(
                                pw, lhsT=wBD4k[:, 128 * s:128 * (s + 1)],
                                rhs=gch, start=_ss[0], stop=_ss[1],
                                skip_group_check=True)
                        if True:
                            dist = chain.tile([128, TF], f16, tag="dist")
                            nc.scalar.activation(dist, pc, Act.Sqrt)
                            h = chain.tile([128, TF], f16, tag="h")
                            nc.scalar.activation(h, dist, Act.Relu, bias=negH)
                            h2 = chain.tile([128, TF], f16, tag="h2")
                            nc.vector.tensor_tensor(h2, h, h, op=Alu.mult)
                            # NOTE: tensor_tensor_reduce wedges real hardware
                            # here (NRT_EXEC_UNIT_UNRECOVERABLE); use a plain
                            # mult + reduce instead.
                            h2w = chain.tile([128, TF], f16, tag="h2w")
                            nc.vector.tensor_tensor(h2w, h2, pw, op=Alu.mult)
                            nc.vector.reduce_sum(acc_cols[:, G:G + 1], h2w,
                                                 axis=mybir.AxisListType.X)

                var_col = small.tile([128, 1], f32, tag="var_col")
                nc.vector.reduce_sum(var_col, acc_cols,
                                     axis=mybir.AxisListType.X)
                nc.sync.dma_start(out_acc, var_col)

    nc.compile()
    return nc


def _get_nc(reps=1):
    key = ("nc", reps)
    if key not in _CACHE:
        _CACHE[key] = _build(reps)
    return _CACHE[key]


def _host_combine(accs, means_all, n_objects):
    """Per-core device outputs -> final scalar loss (float64 on host)."""
    losses = []
    for b in range(BS):
        no = float(n_objects[b])
        valid = (np.arange(NI) < n_objects[b]).astype(np.float64)

        var_term = float(accs[b].astype(np.float64).sum() / WSCALE / no)

        means = means_all[b].astype(np.float64) * valid[:, None]
        diff = means[:, None, :] - means[None, :, :]
        psq = np.clip((diff * diff).sum(-1), EPS, None)
        pnorm = np.sqrt(psq)
        eye = np.eye(NI)
        margin = 2.0 * DELTA_DIST * (1.0 - eye)
        pair_mask = valid[:, None] * valid[None, :] * (1.0 - eye)
        hinge = np.clip(margin - pnorm, 0.0, None) ** 2 * pair_mask
        denom = max(no * (no - 1.0), 1.0)
        multi = 1.0 if n_objects[b] > 1 else 0.0
        dist_term = float(hinge.sum() / denom * multi)

        mnorm = np.sqrt(np.clip((means * means).sum(-1), EPS, None)) * valid
        reg_term = float(mnorm.sum() / no)

        losses.append(VAR_W * var_term + DIST_W * dist_term + REG_W * reg_term)
    return np.float32(np.mean(losses))


def _run(prediction, target, n_objects, trace=False, reps=1, **spmd_kwargs):
    from concourse.bass_utils import run_bass_kernel_spmd

    nc = _get_nc(reps)
    consts = _host_consts()

    pred = np.ascontiguousarray(np.asarray(prediction, dtype=np.float32))
    targ = np.ascontiguousarray(np.asarray(target, dtype=np.float32))
    nobj = np.asarray(n_objects)

    in_maps = []
    for b in range(BS):
        m = _host_prep(pred[b].reshape(16, L), targ[b].reshape(16, L))
        m.update(consts)
        in_maps.append(m)

    res = run_bass_kernel_spmd(nc, in_maps, list(range(N_CORES)),
                               trace=trace, **spmd_kwargs)
    accs = [res.results[b]["out_acc"] for b in range(BS)]
    means = [res.results[b]["out_means"] for b in range(BS)]
    return _host_combine(accs, means, nobj), res


def kernel(prediction, target, n_objects):
    loss, _ = _run(prediction, target, n_objects)
    return loss
